# revision 25
# baseline (speedup 1.0000x reference)
"""Trainium2 Bass kernel for nn_DefinableVectorRouting.

Strategy: shard n_inp (i) 8 ways across cores. Each core owns i-rows
[128c, 128c+128), computes iter0 partial x_out0 in h-partitioned layout,
folds W_G immediately (pred partial), one 8MB fp32 AllReduce of pred
partials, then S/softmax/phi locally and iter1 partial x_out1. Host sums
the 8 partial outputs.

Host-side prep (layout only): x_inp transposed per core to [d, (b, i)]
so all d-contractions have d on SBUF partitions.
"""

import os
import sys

import numpy as np

for _p in ("/opt/trn_rl_repo",):
    if _p not in sys.path:
        sys.path.insert(0, _p)

N_CORES = 8
B, N_INP, D_INP, N_OUT, D_OUT = 32, 1024, 1024, 64, 1024
ISH = N_INP // N_CORES  # 128 i-rows per core
DT = D_INP // 128  # 8 d-tiles
HT = D_OUT // 128  # 8 h-tiles
SIM_SCALE = float(D_INP) ** -0.5
JB = N_OUT * B  # 2048 flattened (j, b)

# dtype knobs (switch to probe precision/perf tradeoffs)
ITER1_F32R = os.environ.get("K_ITER1_F32R", "0") == "1"


def build_kernel_body(tc, ins, outs):
    import concourse.bass as bass
    import concourse.mybir as mybir

    dt = mybir.dt
    f32, bf16 = dt.float32, dt.bfloat16
    AF = mybir.ActivationFunctionType
    ALU = mybir.AluOpType

    nc = tc.nc
    xT = ins["xT"]  # [1024 d, 4096 (b,i)] fp32, d-major tiles
    wmem = ins["wmem"]  # [128 i, 64 j, 1024 h] fp32
    wg = ins["wg"]  # [1024 h, 1024 d] fp32
    wa = ins["wa"]  # [1024] fp32
    ba = ins["ba"]  # [1] fp32
    bg = ins["bg"]  # [1024] fp32
    bu = ins["bu"]  # [128 i, 64 j] fp32
    bi = ins["bi"]  # [128 i, 64 j] fp32
    out = outs["out"]  # [32 b, 64 j, 1024 h] fp32 (partial; host sums cores)

    from contextlib import ExitStack

    ctx = ExitStack()
    with ctx:
        # ---------------- constants ----------------
        const = ctx.enter_context(tc.tile_pool(name="const", bufs=1))
        wa_sb = const.tile([128, DT], f32)
        nc.sync.dma_start(out=wa_sb, in_=wa)
        ba_sb = const.tile([128, 1], f32)
        nc.sync.dma_start(out=ba_sb, in_=ba.to_broadcast([128, 1]))
        bg_sb = const.tile([128, DT], f32)
        nc.sync.dma_start(out=bg_sb, in_=bg)
        bu_sb = const.tile([128, N_OUT], f32)
        nc.sync.dma_start(out=bu_sb, in_=bu)
        bi_sb = const.tile([128, N_OUT], f32)
        nc.sync.dma_start(out=bi_sb, in_=bi)
        # c0 = bu/64 - bi*(63/64);  bsum = bu + bi
        c0_sb = const.tile([128, N_OUT], f32)
        c0_tmp = const.tile([128, N_OUT], f32)
        nc.vector.tensor_scalar_mul(c0_sb, bu_sb, 1.0 / N_OUT)
        nc.vector.tensor_scalar_mul(c0_tmp, bi_sb, (N_OUT - 1.0) / N_OUT)
        nc.vector.tensor_sub(c0_sb, c0_sb, c0_tmp)
        bsum_sb = const.tile([128, N_OUT], f32)
        nc.vector.tensor_add(bsum_sb, bu_sb, bi_sb)

        # W_G -> bf16 resident [ht][128 h, 1024 d]
        wg_bf = const.tile([128, HT, D_INP], bf16)
        for ht in range(HT):
            nc.gpsimd.dma_start(
                out=wg_bf[:, ht, :], in_=wg[ht * 128:(ht + 1) * 128, :]
            )

        # x resident bf16 [128 d, dt, 4096 (b,i)]
        x_bf = const.tile([128, DT, B * ISH], bf16)

        # collective bounce buffers
        dram = ctx.enter_context(tc.tile_pool(name="dram", bufs=1, space="DRAM"))
        pred_par_d = dram.tile([D_INP, JB], f32)
        pred_full_d = dram.tile([D_INP, JB], f32, addr_space="Shared")

        # f_a^T [128 i, 32 b] fp32
        fa_t = const.tile([128, B], f32)
        # phi0 (iter0 lhs-side moving operand) bf16 [128 i, (j,b)]
        phi0_bf = const.tile([128, JB], bf16)

        # ---------------- phase B: load x, f_a matvec ----------------
        # a^T[i, b] = sum_d x^T[d,(b,i)] * W_A[d]: x tile slice [d, i] as
        # stationary lhsT (M=i), W_A d-tile column as moving rhs (N=1),
        # accumulating over d-tiles directly into psum column b.
        with nc.named_scope("fa"), \
             tc.tile_pool(name="xf32", bufs=2) as xf32_pool, \
             tc.tile_pool(name="fa_ps", bufs=2, space="PSUM") as fa_ps_pool, \
             tc.tile_pool(name="fa_acc", bufs=1) as fa_acc_pool:
            a_acc = fa_acc_pool.tile([128, B], f32)
            nc.vector.memset(a_acc, 0.0)
            for t in range(DT):
                x_f32 = xf32_pool.tile([128, B * ISH], f32, tag="xf")
                nc.sync.dma_start(
                    out=x_f32, in_=xT[t * 128:(t + 1) * 128, :]
                )
                # cast to resident bf16 (ScalarE so DVE stays free)
                nc.scalar.copy(out=x_bf[:, t, :], in_=x_f32)
                # per-(b,t) single-shot matmuls into a per-t psum tile;
                # accumulate across t on DVE (interleaved psum groups in
                # one bank are not allowed)
                a_ps = fa_ps_pool.tile([128, B], f32, tag="aps")
                for b in range(B):
                    nc.tensor.matmul(
                        a_ps[:, b:b + 1],
                        lhsT=x_f32[:, b * ISH:(b + 1) * ISH],
                        rhs=wa_sb[:, t:t + 1],
                        start=True, stop=True,
                    )
                nc.vector.tensor_add(a_acc, a_acc, a_ps)
            # f_a^T = sigmoid(a^T + b_A)
            nc.scalar.activation(
                out=fa_t, in_=a_acc, func=AF.Sigmoid, bias=ba_sb,
            )

        # phi0[i, (j,b)] = f_a[i,b] * c0[i,j]  (broadcast tensor_tensor)
        fa_bc = bass.AP(
            tensor=fa_t.tensor, offset=fa_t.offset,
            ap=[fa_t.ap[0], [0, N_OUT], fa_t.ap[1]],
        )
        c0_bc = bass.AP(
            tensor=c0_sb.tensor, offset=c0_sb.offset,
            ap=[c0_sb.ap[0], c0_sb.ap[1], [0, B]],
        )
        nc.vector.tensor_tensor(
            phi0_bf.rearrange("p (j b) -> p j b", j=N_OUT), fa_bc, c0_bc, ALU.mult
        )

        # ---------------- phase C+D: iter0 (W_mem stationary) + pred ----------------
        # xo_T[ht][128 h, (j,b)] bf16 : x_out0 partial, h-partitioned
        with nc.named_scope("it0pred"), \
             tc.tile_pool(name="xo", bufs=1) as xo_pool, \
             tc.tile_pool(name="pred_sb", bufs=4) as pred_sb_pool:
            xo_t = xo_pool.tile([128, HT, JB], bf16)
            JG = 16  # j-group size for psum banking
            with tc.tile_pool(name="wm1", bufs=4) as wm1_pool, \
                 tc.tile_pool(name="it0_ps", bufs=1, space="PSUM") as it0_ps_pool:
                for g in range(N_OUT // JG):
                    ps_tiles = [
                        it0_ps_pool.tile([128, JG * B], f32, tag=f"it0_{ht}",
                                         name=f"it0ps{ht}")
                        for ht in range(HT)
                    ]
                    for jj in range(JG):
                        j = g * JG + jj
                        wm_bf = wm1_pool.tile([128, D_OUT], bf16, tag="wm1")
                        # cast-DMA fp32 HBM -> bf16 SBUF (SWDGE)
                        nc.gpsimd.dma_start(out=wm_bf, in_=wmem[:, j, :])
                        for ht in range(HT):
                            nc.tensor.matmul(
                                ps_tiles[ht][:, jj * B:(jj + 1) * B],
                                lhsT=wm_bf[:, ht * 128:(ht + 1) * 128],
                                rhs=phi0_bf[:, j * B:(j + 1) * B],
                                start=True, stop=True,
                            )
                    for ht in range(HT):
                        nc.vector.tensor_copy(
                            out=xo_t[:, ht, g * JG * B:(g + 1) * JG * B],
                            in_=ps_tiles[ht],
                        )
            # pred partial: [d, (j,b)] = sum_h W_G[h,d] * xo_T[h,(j,b)]
            NCH = JB // 512  # 4
            with tc.tile_pool(name="pred_ps", bufs=2, space="PSUM") as pred_ps_pool:
                for t in range(DT):
                    for chn in range(NCH):
                        p_ps = pred_ps_pool.tile([128, 512], f32, tag="pred")
                        for ht in range(HT):
                            nc.tensor.matmul(
                                p_ps,
                                lhsT=wg_bf[:, ht, t * 128:(t + 1) * 128],
                                rhs=xo_t[:, ht, chn * 512:(chn + 1) * 512],
                                start=(ht == 0),
                                stop=(ht == HT - 1),
                            )
                        p_sb = pred_sb_pool.tile([128, 512], f32, tag="psb")
                        nc.vector.tensor_copy(out=p_sb, in_=p_ps)
                        nc.sync.dma_start(
                            out=pred_par_d[
                                t * 128:(t + 1) * 128, chn * 512:(chn + 1) * 512
                            ],
                            in_=p_sb,
                        )

        # ---------------- phase E: AllReduce pred partials ----------------
        with nc.named_scope("allreduce"):
            nc.gpsimd.collective_compute(
                "AllReduce",
                mybir.AluOpType.add,
                replica_groups=[list(range(N_CORES))],
                ins=[pred_par_d.opt()],
                outs=[pred_full_d.opt()],
            )

        # ---------------- phase F: pred -> sbuf bf16 (+b_G) ----------------
        sm = ctx.enter_context(tc.tile_pool(name="sm", bufs=1))
        pred_bf = sm.tile([128, DT, JB], bf16)
        with nc.named_scope("predback"), \
             tc.tile_pool(name="predf", bufs=2) as predf_pool:
            for t in range(DT):
                pf = predf_pool.tile([128, JB], f32, tag="pf")
                nc.sync.dma_start(out=pf, in_=pred_full_d[t * 128:(t + 1) * 128, :])
                nc.vector.tensor_scalar_add(
                    pred_bf[:, t, :], pf, bg_sb[:, t:t + 1]
                )

        # ---------------- phase G: S + softmax + phi ----------------
        expS = sm.tile([128, B, N_OUT], f32)  # [i, b, j]
        sums = sm.tile([128, B], f32)
        recipf = sm.tile([128, B], f32)
        phi = sm.tile([128, JB], f32)  # [i, (j,b)] j-major
        phi_v = phi.rearrange("p (j b) -> p b j", b=B)
        _sp_id, _ = nc.enter_named_scope("sphase", False)
        with tc.tile_pool(name="s_ps", bufs=4, space="PSUM") as s_ps_pool:
            for bb in range(B // 8):
                s_ps = s_ps_pool.tile([128, 8 * N_OUT], f32, tag="sps")
                for k in range(8):
                    b = bb * 8 + k
                    rhs = pred_bf.rearrange("p t (j b) -> p t b j", b=B)
                    for t in range(DT):
                        nc.tensor.matmul(
                            s_ps[:, k * N_OUT:(k + 1) * N_OUT],
                            lhsT=x_bf[:, t, b * ISH:(b + 1) * ISH],
                            rhs=rhs[:, t, b, :],
                            start=(t == 0),
                            stop=(t == DT - 1),
                        )
                # exp(S * sim_scale)
                nc.scalar.activation(
                    out=expS[:, bb * 8:(bb + 1) * 8, :],
                    in_=s_ps,
                    func=AF.Exp,
                    scale=SIM_SCALE,
                )
        nc.vector.tensor_reduce(
            out=sums, in_=expS, axis=mybir.AxisListType.X, op=ALU.add
        )
        nc.vector.reciprocal(out=recipf, in_=sums)
        nc.vector.tensor_mul(recipf, recipf, fa_t)
        # bif[i, b, j] = bi[i,j] * fa[i,b] (broadcast)
        bif = sm.tile([128, B, N_OUT], f32)
        bi_bc = bass.AP(
            tensor=bi_sb.tensor, offset=bi_sb.offset,
            ap=[bi_sb.ap[0], [0, B], bi_sb.ap[1]],
        )
        fa_bc2 = bass.AP(
            tensor=fa_t.tensor, offset=fa_t.offset,
            ap=[fa_t.ap[0], fa_t.ap[1], [0, N_OUT]],
        )
        nc.vector.tensor_tensor(bif, fa_bc2, bi_bc, ALU.mult)
        # Rf = expS * recipf (bcast over j); Rf *= bsum (bcast over b);
        # phi = Rf - bif   (written strided into [i, (j,b)] layout)
        Rf = sm.tile([128, B, N_OUT], f32)
        recipf_bc = bass.AP(
            tensor=recipf.tensor, offset=recipf.offset,
            ap=[recipf.ap[0], recipf.ap[1], [0, N_OUT]],
        )
        bsum_bc = bass.AP(
            tensor=bsum_sb.tensor, offset=bsum_sb.offset,
            ap=[bsum_sb.ap[0], [0, B], bsum_sb.ap[1]],
        )
        nc.vector.tensor_tensor(Rf, expS, recipf_bc, ALU.mult)
        nc.vector.tensor_tensor(Rf, Rf, bsum_bc, ALU.mult)
        nc.vector.tensor_tensor(phi_v, Rf, bif, ALU.subtract)
        nc.leave_named_scope("sphase", _sp_id, False)

        # ---------------- phase I: iter1 ----------------
        it1_dt = mybir.dt.float32r if ITER1_F32R else mybir.dt.float32
        with nc.named_scope("iter1"), \
             tc.tile_pool(name="wm2", bufs=8) as wm2_pool, \
             tc.tile_pool(name="o_ps", bufs=4, space="PSUM") as o_ps_pool, \
             tc.tile_pool(name="o_sb", bufs=3) as o_sb_pool:
            for g in range(N_OUT // 4):  # groups of 4 j, col-packed
                wm_tiles = []
                for jj in range(4):
                    j = g * 4 + jj
                    wm2 = wm2_pool.tile([128, D_OUT], f32, tag="wm2")
                    nc.sync.dma_start(out=wm2, in_=wmem[:, j, :])
                    wm_tiles.append(wm2)
                o_sb = o_sb_pool.tile([128, D_OUT], f32, tag="osb")
                for half in range(2):
                    o_ps = o_ps_pool.tile([128, 512], f32, tag="ops")
                    for jj in range(4):
                        j = g * 4 + jj
                        lhsT = phi[:, j * B:(j + 1) * B]
                        rhs = wm_tiles[jj][:, half * 512:(half + 1) * 512]
                        if ITER1_F32R:
                            lhsT = lhsT.bitcast(it1_dt)
                            rhs = rhs.bitcast(it1_dt)
                        nc.tensor.matmul(
                            o_ps[jj * B:(jj + 1) * B, :],
                            lhsT=lhsT,
                            rhs=rhs,
                            start=True, stop=True,
                            tile_position=(0, jj * B),
                        )
                    nc.vector.tensor_copy(
                        out=o_sb[:, half * 512:(half + 1) * 512], in_=o_ps
                    )
                # DMA out: sbuf rows [jj*32:(jj+1)*32] -> dram out[:, 4g+jj, :]
                for jj in range(4):
                    nc.sync.dma_start(
                        out=out[:, g * 4 + jj, :],
                        in_=o_sb[jj * B:(jj + 1) * B, :],
                    )


def build_bass():
    import concourse.mybir as mybir
    import concourse.tile as tile
    from concourse import bacc

    dt = mybir.dt
    nc = bacc.Bacc(
        "TRN2", target_bir_lowering=False, debug=False, num_devices=N_CORES
    )
    ins = {
        "xT": nc.dram_tensor("xT", [D_INP, B * ISH], dt.float32,
                             kind="ExternalInput").ap(),
        "wmem": nc.dram_tensor("wmem", [ISH, N_OUT, D_OUT], dt.float32,
                               kind="ExternalInput").ap(),
        "wg": nc.dram_tensor("wg", [D_OUT, D_INP], dt.float32,
                             kind="ExternalInput").ap(),
        "wa": nc.dram_tensor("wa", [128, DT], dt.float32,
                             kind="ExternalInput").ap(),
        "ba": nc.dram_tensor("ba", [1], dt.float32, kind="ExternalInput").ap(),
        "bg": nc.dram_tensor("bg", [128, DT], dt.float32,
                             kind="ExternalInput").ap(),
        "bu": nc.dram_tensor("bu", [ISH, N_OUT], dt.float32,
                             kind="ExternalInput").ap(),
        "bi": nc.dram_tensor("bi", [ISH, N_OUT], dt.float32,
                             kind="ExternalInput").ap(),
    }
    outs = {
        "out": nc.dram_tensor("out", [B, N_OUT, D_OUT], dt.float32,
                              kind="ExternalOutput").ap(),
    }
    with tile.TileContext(nc) as tc:
        build_kernel_body(tc, ins, outs)
    nc.compile()
    return nc


def shard_inputs(inputs):
    x_inp = np.asarray(inputs["x_inp"], dtype=np.float32)
    # [1024] -> [128, 8] with element (p, t) = v[t*128 + p]
    W_A = np.ascontiguousarray(
        np.asarray(inputs["W_A"], dtype=np.float32).reshape(DT, 128).T)
    b_A = np.asarray(inputs["b_A"], dtype=np.float32).reshape(1)
    W_mem = np.asarray(inputs["W_mem"], dtype=np.float32)
    W_G = np.asarray(inputs["W_G"], dtype=np.float32)
    b_G = np.ascontiguousarray(
        np.asarray(inputs["b_G"], dtype=np.float32).reshape(DT, 128).T)
    beta_use = np.asarray(inputs["beta_use"], dtype=np.float32)
    beta_ign = np.asarray(inputs["beta_ign"], dtype=np.float32)
    in_maps = []
    for c in range(N_CORES):
        sl = slice(c * ISH, (c + 1) * ISH)
        xc = x_inp[:, sl, :]  # [32, 128, 1024]
        xT = np.ascontiguousarray(xc.transpose(2, 0, 1)).reshape(D_INP, B * ISH)
        in_maps.append({
            "xT": xT,
            "wmem": np.ascontiguousarray(W_mem[sl]),
            "wg": W_G,
            "wa": W_A,
            "ba": b_A,
            "bg": b_G,
            "bu": np.ascontiguousarray(beta_use[sl]),
            "bi": np.ascontiguousarray(beta_ign[sl]),
        })
    return in_maps


_CACHED = {}


def kernel(**inputs) -> np.ndarray:
    from concourse import bass_utils

    if "nc" not in _CACHED:
        _CACHED["nc"] = build_bass()
    nc = _CACHED["nc"]
    in_maps = shard_inputs(inputs)
    trace = os.environ.get("K_TRACE", "0") == "1"
    res = bass_utils.run_bass_kernel_spmd(
        nc, in_maps, core_ids=list(range(N_CORES)), trace=trace
    )
    if trace:
        _CACHED["last_results"] = res
    out = np.zeros((B, N_OUT, D_OUT), dtype=np.float64)
    for c in range(N_CORES):
        out += res.results[c]["out"].astype(np.float64)
    return out.astype(np.float32)


if __name__ == "__main__":
    nc = build_bass()
    print("build OK")
